# revision 16
# baseline (speedup 1.0000x reference)
"""Trainium2 Bass kernel for nn_MAHABlock (multiscale-attention block).

Sharding: 8 cores = 4 batches x 2 query-halves. Each core redundantly
computes the conv front-end + key/value pyramid for its batch (needed in
full by every query), and computes queries / attention / FFN only for its
512-row half. No cross-core communication.

On-chip layout: activations are kept feature-major ([d, t]) so weight
blocks serve as the stationary matmul operand, except where row-major is
required (v for attention, LN row statistics). All large matmuls run as
float32r (TF32-like, full rate at N>=512); attention probabilities and a
few tiny side matmuls use bf16.
"""

import numpy as np
import ml_dtypes

import concourse.bass as bass
import concourse.mybir as mybir
import concourse.tile as tile
from concourse import bacc
from concourse.bass_utils import run_bass_kernel_spmd

F32 = mybir.dt.float32
F32R = mybir.dt.float32r
BF16 = mybir.dt.bfloat16
AF = mybir.ActivationFunctionType
OP = mybir.AluOpType

B, L, D, H, F, S, CR = 4, 1024, 1024, 16, 4096, 4, 2
DH = D // H          # 64
QH = L // 2          # 512 query rows per core
DC = D // 128        # 8
FC = F // 128        # 32
NC = 8
LS = [L >> s for s in range(S)]   # [1024, 512, 256, 128]
EPS = 1e-5

_CACHE = {}


def _build(debug=False):
    nc = bacc.Bacc(None, target_bir_lowering=False)
    with tile.TileContext(nc) as tc:
        dram = tc.alloc_tile_pool(name="dram", bufs=1, space="DRAM")

        def din(name, shape, dt=F32R):
            return dram.tile(shape, dt, kind="ExternalInput", name=name,
                             uniquify=False)

        xt_d = din("xt_pad", [D, L + 4])
        xres_d = din("xres", [QH, D], F32)
        wdc_d = din("wdc", [3, DC, 4, 128, 256])
        bdc_d = din("bdc", [128, DC], F32)
        wdec_d = din("wdec", [2, CR, DC, 4, 128, 256])
        wdec2_d = din("wdec2", [CR, DC, 4, 128, 256], BF16)
        wq_d = din("wq", [S, DC, 4, 128, 256])
        wk_d = din("wk", [S, DC, 4, 128, 256])
        wv_d = din("wv", [DC, 2, 128, 512])
        wo_d = din("wo", [DC, 2, 128, 512])
        w1_d = din("w1", [DC, 16, 128, 256])
        b1_d = din("b1", [128, FC], F32)
        w2_d = din("w2", [FC, 2, 128, 512], BF16)
        b2i_d = din("b2img", [128, D], F32)
        g1i_d = din("g1img", [128, D], F32)
        t1i_d = din("bt1img", [128, D], F32)
        g2i_d = din("g2img", [128, D], F32)
        t2i_d = din("bt2img", [128, D], F32)
        wvec_d = din("wvec", [H, S], F32)
        id_d = din("ident", [128, 128])
        out_d = dram.tile([QH, D], F32, kind="ExternalOutput", name="out",
                          uniquify=False)
        if debug:
            dbg = {}
            for nm, shp in [("dxc", [128, DC, L]), ("dsc1", [128, DC, 512]),
                            ("dsc3", [128, DC, 128]),
                            ("dagg", [128, DC, QH]), ("dx1", [128, 4, D]),
                            ("doun", [128, DC, QH])]:
                dbg[nm] = dram.tile(shp, F32R, kind="ExternalOutput",
                                    name=nm, uniquify=False)

        # ---- constants (live for the whole kernel) ----
        const = tc.alloc_tile_pool(name="const", bufs=1)
        bdc_sb = const.tile([128, DC], F32, name="bdc_sb")
        nc.sync.dma_start(bdc_sb[:], bdc_d[:])
        wvec_sb = const.tile([H, S], F32, name="wvec_sb")
        nc.sync.dma_start(wvec_sb[:], wvec_d[:])
        ident = const.tile([128, 128], F32R, name="ident")
        nc.sync.dma_start(ident[:], id_d[:])
        ones64 = const.tile([1, 64], BF16, name="ones64")
        nc.vector.memset(ones64[:], 1.0)
        zcol = const.tile([128, 1], F32, name="zcol")
        nc.vector.memset(zcol[:], 0.0)
        epscol = const.tile([128, 1], F32, name="epscol")
        nc.vector.memset(epscol[:], EPS)

        # global weight-streaming + psum pools (whole kernel)
        wpool = tc.alloc_tile_pool(name="wpool", bufs=8)
        pspool = tc.alloc_tile_pool(name="pspool", bufs=4, space="PSUM")

        def wt_tile():
            return wpool.tile([128, 256], F32R, name="wt", bufs=8)

        def wtb_tile():
            return wpool.tile([128, 256], BF16, name="wtb", bufs=4)

        def wbig_tile(dt=F32R):
            return wpool.tile([128, 512], dt, name="wbig", bufs=8)

        def ps_t(shape=None):
            return pspool.tile(shape or [128, 512], F32, name="ps_t",
                               bufs=4)

        def ps_s(shape=None, dt=F32):
            return pspool.tile(shape or [128, 512], dt, name="ps_s",
                               bufs=2)

        def ps_o():
            return pspool.tile([65, QH], F32, name="ps_o", bufs=2)

        pid = nc.partition_id()
        q0 = (pid % 2) * QH

        # long-lived activation pools (stack: released near the end)
        pX1 = tc.alloc_tile_pool(name="pX1", bufs=1)
        x1 = pX1.tile([128, 4, D], F32R, name="x1")
        pAgg = tc.alloc_tile_pool(name="pAgg", bufs=1)
        agg = pAgg.tile([128, DC, QH], F32R, name="agg")

        pXC = tc.alloc_tile_pool(name="pXC", bufs=1)
        xc = pXC.tile([128, DC, L], F32R, name="xc")

        # ---------------- Phase A: dilated conv -> xc (feature-major) ---
        pA = tc.alloc_tile_pool(name="pA", bufs=1)
        xt_sb = pA.tile([128, DC, L + 4], F32R, name="xt_sb")
        nc.sync.dma_start(
            xt_sb[:], xt_d[:].rearrange("(c p) t -> p c t", p=128))
        for oj in range(4):
            paccs = [ps_t() for _ in range(4)]
            for k in range(3):
                for ic in range(DC):
                    wt = wt_tile()
                    nc.sync.dma_start(wt[:], wdc_d[k, ic, oj])
                    first = (k == 0 and ic == 0)
                    last = (k == 2 and ic == DC - 1)
                    for half in range(2):
                        for tt in range(2):
                            nc.tensor.matmul(
                                paccs[half * 2 + tt][:],
                                wt[:, half * 128:half * 128 + 128],
                                xt_sb[:, ic, tt * 512 + 2 * k:
                                      tt * 512 + 2 * k + 512],
                                start=first, stop=last)
            for half in range(2):
                oc = oj * 2 + half
                for tt in range(2):
                    nc.scalar.activation(
                        xc[:, oc, tt * 512:tt * 512 + 512],
                        paccs[half * 2 + tt][:], AF.Relu,
                        bias=bdc_sb[:, oc:oc + 1])
        if debug:
            nc.sync.dma_start(dbg["dxc"][:], xc[:])
        pA.release()

        # ---------------- Phase B: pyramid (strided convs) --------------
        pSC = tc.alloc_tile_pool(name="pSC", bufs=1)
        sc1 = pSC.tile([128, DC, LS[1]], F32R, name="sc1")
        sc2 = pSC.tile([128, DC, LS[2]], F32R, name="sc2")
        sc3 = pSC.tile([128, DC, LS[3]], F32R, name="sc3")
        pB = tc.alloc_tile_pool(name="pB", bufs=1)
        srcs = [xc, sc1, sc2]
        dsts = [sc1, sc2, sc3]
        for s in range(3):
            src, dst = srcs[s], dsts[s]
            ln = LS[s + 1]
            if s == 2:
                srcb = pB.tile([128, DC, LS[2]], BF16, name="srcb")
                nc.vector.tensor_copy(srcb[:], sc2[:])
                src = srcb
            for oj in range(4):
                paccs = [ps_t([128, ln]) for _ in range(2)]
                for k in range(CR):
                    for ic in range(DC):
                        if s == 2:
                            wt = wtb_tile()
                            nc.sync.dma_start(wt[:], wdec2_d[k, ic, oj])
                        else:
                            wt = wt_tile()
                            nc.sync.dma_start(wt[:], wdec_d[s, k, ic, oj])
                        first = (k == 0 and ic == 0)
                        last = (k == CR - 1 and ic == DC - 1)
                        for half in range(2):
                            nc.tensor.matmul(
                                paccs[half][:],
                                wt[:, half * 128:half * 128 + 128],
                                src[:, ic, k:2 * ln:2],
                                start=first, stop=last)
                for half in range(2):
                    nc.vector.tensor_copy(dst[:, oj * 2 + half, :],
                                          paccs[half][:])
        if debug:
            nc.sync.dma_start(dbg["dsc1"][:], sc1[:])
            nc.sync.dma_start(dbg["dsc3"][:], sc3[:])
        pB.release()

        # ---------------- Phase C: attention over 4 scales --------------
        pCt = tc.alloc_tile_pool(name="pCt", bufs=1)
        probsC = tc.alloc_tile_pool(name="probsC", bufs=4)
        tmpC = tc.alloc_tile_pool(name="tmpC", bufs=2)
        for s in range(S):
            src = [xc, sc1, sc2, sc3][s]
            ls = LS[s]
            kk_n = ls // 128
            nt_n = (ls + 511) // 512
            oun = pCt.tile([128, DC, QH], F32, name="oun")
            wv_tiles = None
            if debug and s == 0:
                dbg_oun_pending = True
            for g in range(4):      # head groups: heads 4g..4g+3
                oc0 = 2 * g        # d-chunks 2g, 2g+1 <-> q/k rows
                # --- queries for this group's chunks
                qs = pCt.tile([128, 2, QH], F32R, name="qs", bufs=2)
                pq = [ps_t([128, QH]) for _ in range(2)]
                for ic in range(DC):
                    wtq = wt_tile()
                    nc.sync.dma_start(wtq[:], wq_d[s, ic, g])
                    for half in range(2):
                        nc.tensor.matmul(
                            pq[half][:],
                            wtq[:, half * 128:half * 128 + 128],
                            xc[:, ic, bass.ds(q0, QH)],
                            start=(ic == 0), stop=(ic == DC - 1))
                for half in range(2):
                    nc.vector.tensor_copy(qs[:, half, :], pq[half][:])
                # --- keys (feature-major) for chunks 2g, 2g+1
                kt = pCt.tile([128, 2, ls], F32R, name="kt")
                for nt in range(nt_n):
                    n = min(512, ls - nt * 512)
                    pk = [ps_t([128, n]) for _ in range(2)]
                    for ic in range(DC):
                        wtk = wt_tile()
                        nc.sync.dma_start(wtk[:], wk_d[s, ic, g])
                        for half in range(2):
                            nc.tensor.matmul(
                                pk[half][:],
                                wtk[:, half * 128:half * 128 + 128],
                                src[:, ic, nt * 512:nt * 512 + n],
                                start=(ic == 0), stop=(ic == DC - 1))
                    for half in range(2):
                        nc.vector.tensor_copy(
                            kt[:, half, nt * 512:nt * 512 + n],
                            pk[half][:])
                # --- values (row-major, bf16, interleaved ones column)
                if g % 2 == 0:
                    wv_tiles = []
                    for ic in range(DC):
                        wt = wbig_tile()
                        nc.sync.dma_start(wt[:], wv_d[ic, g // 2])
                        wv_tiles.append(wt)
                va = pCt.tile([128, kk_n, 4 * 65], BF16, name="va", bufs=2)
                nc.vector.memset(va[:, :, 64:4 * 65:65], 1.0)
                for kk in range(kk_n):
                    pacc = ps_t([128, 256])
                    for ic in range(DC):
                        nc.tensor.matmul(
                            pacc[:], src[:, ic, kk * 128:kk * 128 + 128],
                            wv_tiles[ic][:,
                                         (g % 2) * 256:(g % 2) * 256 + 256],
                            start=(ic == 0), stop=(ic == DC - 1))
                    nc.vector.tensor_copy(
                        va[:, kk, :].rearrange("p (h c) -> p h c", c=65)
                        [:, :, 0:64],
                        pacc[:].rearrange("p (h c) -> p h c", c=64))
                # --- attention for the 4 heads
                pr = None
                for hh in range(4):
                    half, off = hh // 2, (hh % 2) * 64
                    pov = ps_o()
                    for kk in range(kk_n):
                        pss = ps_s([128, QH])
                        nc.tensor.matmul(
                            pss[:],
                            kt[off:off + 64, half, kk * 128:kk * 128 + 128],
                            qs[off:off + 64, half, :],
                            start=True, stop=True)
                        pb = probsC.tile([128, QH], BF16, name="pb")
                        nc.scalar.activation(pb[:], pss[:], AF.Exp,
                                             bias=zcol[:],
                                             scale=1.0 / np.sqrt(DH))
                        nc.tensor.matmul(
                            pov[0:65, :], va[:, kk, hh * 65:hh * 65 + 65],
                            pb[:], start=(kk == 0), stop=(kk == kk_n - 1))
                    nc.vector.tensor_copy(
                        oun[off:off + 64, oc0 + half, :], pov[0:64, :])
                    rwcol = tmpC.tile([1, QH], F32, name="rwcol")
                    nc.vector.reciprocal(rwcol[:], pov[64:65, :])
                    nc.vector.tensor_scalar_mul(rwcol[:], rwcol[:],
                                                wvec_sb[0:1, s:s + 1])
                    rwbcol = tmpC.tile([1, QH], BF16, name="rwbcol")
                    nc.vector.tensor_copy(rwbcol[:], rwcol[:])
                    if hh % 2 == 0:
                        pr = pspool.tile([128, QH], F32, name="ps_o",
                                         bufs=2)
                    nc.tensor.matmul(pr[off:off + 64, :], ones64[0:1, :],
                                     rwbcol[0:1, :], start=True, stop=True)
                    if hh % 2 == 1:
                        oc = oc0 + half
                        if s == 0:
                            nc.vector.tensor_tensor(
                                out=agg[:, oc, :], in0=oun[:, oc, :],
                                in1=pr[:], op=OP.mult)
                        else:
                            t = tmpC.tile([128, QH], F32, name="tagg")
                            nc.vector.tensor_tensor(
                                out=t[:], in0=oun[:, oc, :],
                                in1=pr[:], op=OP.mult)
                            nc.vector.tensor_tensor(
                                out=agg[:, oc, :], in0=agg[:, oc, :],
                                in1=t[:], op=OP.add)
            if debug and s == 0:
                nc.gpsimd.dma_start(dbg["doun"][:], oun[:])
        if debug:
            nc.sync.dma_start(dbg["dagg"][:], agg[:])
        tmpC.release()
        probsC.release()
        pCt.release()
        pSC.release()
        pXC.release()

        # ---------------- LN machinery + late constants -----------------
        pImgs = tc.alloc_tile_pool(name="pImgs", bufs=1)
        imgs = {}
        for nm, d in [("b2", b2i_d), ("g1", g1i_d), ("t1", t1i_d),
                      ("g2", g2i_d), ("t2", t2i_d)]:
            t = pImgs.tile([128, D], F32, name=f"img_{nm}")
            nc.sync.dma_start(t[:], d[:])
            imgs[nm] = t
        b1_sb = pImgs.tile([128, FC], F32, name="b1_sb")
        nc.sync.dma_start(b1_sb[:], b1_d[:])
        pLN = tc.alloc_tile_pool(name="pLN", bufs=2)
        tmpLN = tc.alloc_tile_pool(name="tmpLN", bufs=3)

        def layer_norm(xin, xout, g_img, t_img):
            # xin/xout: [128, D] APs; row-wise LN along free dim
            musum = tmpLN.tile([128, 1], F32, name="lnmu")
            nc.vector.reduce_sum(musum[:], xin, axis=mybir.AxisListType.X)
            negmu = tmpLN.tile([128, 1], F32, name="lnnm")
            nc.scalar.mul(negmu[:], musum[:], -1.0 / D)
            xm = pLN.tile([128, D], F32, name="lnxm")
            nc.scalar.activation(xm[:], xin, AF.Identity, bias=negmu[:])
            sq = pLN.tile([128, D], F32, name="lnsq")
            ssq = tmpLN.tile([128, 1], F32, name="lnss")
            nc.scalar.activation(sq[:], xm[:], AF.Square, bias=zcol[:],
                                 accum_out=ssq[:])
            std = tmpLN.tile([128, 1], F32, name="lnsd")
            nc.scalar.activation(std[:], ssq[:], AF.Sqrt, scale=1.0 / D,
                                 bias=epscol[:])
            rstd = tmpLN.tile([128, 1], F32, name="lnrs")
            nc.vector.reciprocal(rstd[:], std[:])
            t = pLN.tile([128, D], F32, name="lnt")
            nc.vector.scalar_tensor_tensor(
                out=t[:], in0=xm[:], scalar=rstd[:, 0:1], in1=g_img[:],
                op0=OP.mult, op1=OP.mult)
            nc.vector.tensor_tensor(out=xout, in0=t[:], in1=t_img[:],
                                    op=OP.add)

        # ---------------- Phase D: Wo + residual + LN1 ------------------
        pD = tc.alloc_tile_pool(name="pD", bufs=1)
        xres_sb = pD.tile([128, 4, D], F32, name="xres_sb")
        nc.sync.dma_start(
            xres_sb[:], xres_d[:].rearrange("(c p) d -> p c d", p=128))
        xpms = [pD.tile([128, D], F32, name=f"xpm{i}") for i in range(4)]
        for os_ in range(2):
            pms = [ps_t() for _ in range(4)]
            for dc in range(DC):
                wt = wbig_tile()
                nc.sync.dma_start(wt[:], wo_d[dc, os_])
                for qc in range(4):
                    nc.tensor.matmul(
                        pms[qc][:], agg[:, dc, qc * 128:qc * 128 + 128],
                        wt[:], start=(dc == 0), stop=(dc == DC - 1))
            for qc in range(4):
                nc.vector.tensor_tensor(
                    out=xpms[qc][:, os_ * 512:os_ * 512 + 512],
                    in0=pms[qc][:],
                    in1=xres_sb[:, qc, os_ * 512:os_ * 512 + 512],
                    op=OP.add)
        for qc in range(4):
            layer_norm(xpms[qc][:], x1[:, qc, :], imgs["g1"], imgs["t1"])
        if debug:
            nc.sync.dma_start(dbg["dx1"][:], x1[:])
        pD.release()

        # ---------------- Phase E+F: transpose, FFN ---------------------
        pEF = tc.alloc_tile_pool(name="pEF", bufs=1)
        x1t = pEF.tile([128, DC, QH], F32R, name="x1t")
        for qc in range(4):
            for dc in range(DC):
                pt = ps_s([128, 128], F32R)
                nc.tensor.transpose(pt[:], x1[:, qc, dc * 128:dc * 128 + 128],
                                    ident[:])
                nc.vector.tensor_copy(x1t[:, dc, qc * 128:qc * 128 + 128],
                                      pt[:])

        hT = pEF.tile([128, FC, QH], BF16, name="hT")
        for fj in range(16):
            paccs = [ps_t([128, QH]) for _ in range(2)]
            for ic in range(DC):
                wt = wt_tile()
                nc.sync.dma_start(wt[:], w1_d[ic, fj])
                for half in range(2):
                    nc.tensor.matmul(
                        paccs[half][:], wt[:, half * 128:half * 128 + 128],
                        x1t[:, ic, :], start=(ic == 0), stop=(ic == DC - 1))
            for half in range(2):
                fc = fj * 2 + half
                nc.scalar.activation(hT[:, fc, :], paccs[half][:], AF.Gelu,
                                     bias=b1_sb[:, fc:fc + 1])

        pF2 = tc.alloc_tile_pool(name="pF2", bufs=1)
        tfF = tc.alloc_tile_pool(name="tfF", bufs=2)
        x2s = [pF2.tile([128, D], F32, name=f"x2_{i}") for i in range(4)]
        for os_ in range(2):
            pfs = [ps_t() for _ in range(4)]
            for fc in range(FC):
                wt = wbig_tile(BF16)
                nc.sync.dma_start(wt[:], w2_d[fc, os_])
                for qc in range(4):
                    nc.tensor.matmul(
                        pfs[qc][:], hT[:, fc, qc * 128:qc * 128 + 128],
                        wt[:], start=(fc == 0), stop=(fc == FC - 1))
            for qc in range(4):
                t = tfF.tile([128, 512], F32, name="tf")
                nc.vector.tensor_tensor(
                    out=t[:], in0=pfs[qc][:],
                    in1=imgs["b2"][:, os_ * 512:os_ * 512 + 512], op=OP.add)
                nc.vector.tensor_tensor(
                    out=x2s[qc][:, os_ * 512:os_ * 512 + 512], in0=t[:],
                    in1=x1[:, qc, os_ * 512:os_ * 512 + 512], op=OP.add)
        for qc in range(4):
            outq = tfF.tile([128, D], F32, name="outq")
            layer_norm(x2s[qc][:], outq[:], imgs["g2"], imgs["t2"])
            nc.sync.dma_start(
                out_d[:].rearrange("(c p) d -> p c d", p=128)[:, qc, :],
                outq[:])

        for p in (tfF, pF2, pEF, tmpLN, pLN, pImgs, pAgg, pX1,
                  pspool, wpool, const, dram):
            p.release()
    nc.compile()
    return nc


def _chunk_lhst(w, dt=np.float32):
    # [I, O] -> [I/128, O/256, 128, 256] contiguous blocks
    i, o = w.shape
    return np.ascontiguousarray(
        w.reshape(i // 128, 128, o // 256, 256).transpose(0, 2, 1, 3)
    ).astype(dt, copy=False)


def _chunk_rhs(w):
    # [I, O] -> [I/128, O/512, 128, 512]
    i, o = w.shape
    return np.ascontiguousarray(
        w.reshape(i // 128, 128, o // 512, 512).transpose(0, 2, 1, 3))


def _chunk_rhs_bf16(w):
    return _chunk_rhs(w).astype(ml_dtypes.bfloat16)


def _prep(inputs):
    f = {k: np.asarray(v, dtype=np.float32) for k, v in inputs.items()}
    W_dc, W_dec = f["W_dc"], f["W_dec"]
    wdc = np.stack([_chunk_lhst(W_dc[:, :, k].T) for k in range(3)])
    wdec = np.stack([
        np.stack([_chunk_lhst(W_dec[s, :, :, k].T) for k in range(CR)])
        for s in range(2)])
    wdec2 = np.stack([
        _chunk_lhst(W_dec[2, :, :, k].T, ml_dtypes.bfloat16)
        for k in range(CR)])
    wq = np.stack([_chunk_lhst(f["Wq"][s]) for s in range(S)])
    wk = np.stack([_chunk_lhst(f["Wk"][s]) for s in range(S)])

    ex = np.exp(f["agg_logits"] - f["agg_logits"].max())
    w = (ex / ex.sum()).astype(np.float32)
    aux = np.float32(-(w * np.log(w + np.float32(1e-9))).sum())

    shared = {
        "wdc": wdc,
        "bdc": np.ascontiguousarray(f["b_dc"].reshape(DC, 128).T),
        "wdec": wdec, "wdec2": wdec2, "wq": wq, "wk": wk,
        "wv": _chunk_rhs(f["Wv"]), "wo": _chunk_rhs(f["Wo"]),
        "w1": _chunk_lhst(f["W1"]),
        "b1": np.ascontiguousarray(f["b1"].reshape(FC, 128).T),
        "w2": _chunk_rhs_bf16(f["W2"]),
        "b2img": np.ascontiguousarray(
            np.broadcast_to(f["b2"], (128, D))),
        "g1img": np.ascontiguousarray(np.broadcast_to(f["gamma1"], (128, D))),
        "bt1img": np.ascontiguousarray(np.broadcast_to(f["beta1"], (128, D))),
        "g2img": np.ascontiguousarray(np.broadcast_to(f["gamma2"], (128, D))),
        "bt2img": np.ascontiguousarray(np.broadcast_to(f["beta2"], (128, D))),
        "wvec": np.ascontiguousarray(np.broadcast_to(w, (H, S))),
        "ident": np.eye(128, dtype=np.float32),
    }
    in_maps = []
    x = f["x"]
    for c in range(NC):
        b, half = c // 2, c % 2
        xt = np.zeros((D, L + 4), np.float32)
        xt[:, 2:L + 2] = x[b].T
        m = dict(shared)
        m["xt_pad"] = xt
        m["xres"] = np.ascontiguousarray(x[b, half * QH:(half + 1) * QH, :])
        in_maps.append(m)
    return in_maps, aux


def run(inputs, trace=False, debug=False):
    key = ("ncd" if debug else "nc")
    if key not in _CACHE:
        _CACHE[key] = _build(debug=debug)
    nc = _CACHE[key]
    in_maps, aux = _prep(inputs)
    res = run_bass_kernel_spmd(nc, in_maps, core_ids=list(range(NC)),
                               trace=trace)
    out = np.empty((B, L, D), np.float32)
    for c in range(NC):
        b, half = c // 2, c % 2
        out[b, half * QH:(half + 1) * QH, :] = res.results[c]["out"]
    return (out, aux), res


def kernel(**inputs):
    (out, aux), _ = run(inputs, trace=False)
    return out, aux


# revision 18
# speedup vs baseline: 1.0774x; 1.0774x over previous
"""Trainium2 Bass kernel for nn_MAHABlock (multiscale-attention block).

Sharding: 8 cores = 4 batches x 2 query-halves. Each core redundantly
computes the conv front-end + key/value pyramid for its batch (needed in
full by every query), and computes queries / attention / FFN only for its
512-row half. No cross-core communication.

On-chip layout: activations are kept feature-major ([d, t]) so weight
blocks serve as the stationary matmul operand, except where row-major is
required (v for attention, LN row statistics). All large matmuls run as
float32r (TF32-like, full rate at N>=512); attention probabilities and a
few tiny side matmuls use bf16.
"""

import numpy as np
import ml_dtypes

import concourse.bass as bass
import concourse.mybir as mybir
import concourse.tile as tile
from concourse import bacc
from concourse.bass_utils import run_bass_kernel_spmd

F32 = mybir.dt.float32
F32R = mybir.dt.float32r
BF16 = mybir.dt.bfloat16
AF = mybir.ActivationFunctionType
OP = mybir.AluOpType

B, L, D, H, F, S, CR = 4, 1024, 1024, 16, 4096, 4, 2
DH = D // H          # 64
QH = L // 2          # 512 query rows per core
DC = D // 128        # 8
FC = F // 128        # 32
NC = 8
LS = [L >> s for s in range(S)]   # [1024, 512, 256, 128]
EPS = 1e-5

_CACHE = {}


def _build(debug=False):
    nc = bacc.Bacc(None, target_bir_lowering=False)
    with tile.TileContext(nc) as tc:
        dram = tc.alloc_tile_pool(name="dram", bufs=1, space="DRAM")

        def din(name, shape, dt=F32R):
            return dram.tile(shape, dt, kind="ExternalInput", name=name,
                             uniquify=False)

        xt_d = din("xt_pad", [D, L + 4])
        xres_d = din("xres", [QH, D], F32)
        wdc_d = din("wdc", [3, DC, 4, 128, 256])
        bdc_d = din("bdc", [128, DC], F32)
        wdec_d = din("wdec", [2, CR, DC, 4, 128, 256])
        wdec2_d = din("wdec2", [CR, DC, 4, 128, 256], BF16)
        wq_d = din("wq", [S, DC, 4, 128, 256])
        wk_d = din("wk", [S, DC, 4, 128, 256])
        wv_d = din("wv", [DC, 2, 128, 512], BF16)
        wo_d = din("wo", [DC, 2, 128, 512])
        w1_d = din("w1", [DC, 16, 128, 256], BF16)
        b1_d = din("b1", [128, FC], F32)
        w2_d = din("w2", [FC, 2, 128, 512], BF16)
        b2i_d = din("b2img", [128, D], F32)
        g1i_d = din("g1img", [128, D], F32)
        t1i_d = din("bt1img", [128, D], F32)
        g2i_d = din("g2img", [128, D], F32)
        t2i_d = din("bt2img", [128, D], F32)
        wvec_d = din("wvec", [H, S], F32)
        lnw_d = din("lnwimg", [128, S], F32)
        id_d = din("ident", [128, 128])
        out_d = dram.tile([QH, D], F32, kind="ExternalOutput", name="out",
                          uniquify=False)
        if debug:
            dbg = {}
            for nm, shp in [("dxc", [128, DC, L]), ("dsc1", [128, DC, 512]),
                            ("dsc3", [128, DC, 128]),
                            ("dagg", [128, DC, QH]), ("dx1", [128, 4, D]),
                            ("doun", [128, DC, QH])]:
                dbg[nm] = dram.tile(shp, F32R, kind="ExternalOutput",
                                    name=nm, uniquify=False)

        # ---- constants (live for the whole kernel) ----
        const = tc.alloc_tile_pool(name="const", bufs=1)
        bdc_sb = const.tile([128, DC], F32, name="bdc_sb")
        nc.sync.dma_start(bdc_sb[:], bdc_d[:])
        lnw_sb = const.tile([128, S], F32, name="lnw_sb")
        nc.sync.dma_start(lnw_sb[:], lnw_d[:])
        ident = const.tile([128, 128], F32R, name="ident")
        nc.sync.dma_start(ident[:], id_d[:])
        zcol = const.tile([128, 1], F32, name="zcol")
        nc.vector.memset(zcol[:], 0.0)
        epscol = const.tile([128, 1], F32, name="epscol")
        nc.vector.memset(epscol[:], EPS)

        # global weight-streaming + psum pools (whole kernel)
        wpool = tc.alloc_tile_pool(name="wpool", bufs=8)
        pspool = tc.alloc_tile_pool(name="pspool", bufs=4, space="PSUM")

        def wt_tile():
            return wpool.tile([128, 256], F32R, name="wt", bufs=8)

        def wtb_tile():
            return wpool.tile([128, 256], BF16, name="wtb", bufs=4)

        def wbig_tile(dt=F32R):
            return wpool.tile([128, 512], dt, name="wbig", bufs=8)

        def ps_t(shape=None):
            return pspool.tile(shape or [128, 512], F32, name="ps_t",
                               bufs=4)

        def ps_s(shape=None, dt=F32):
            return pspool.tile(shape or [128, 512], dt, name="ps_s",
                               bufs=2)

        def ps_o():
            return pspool.tile([128, QH], F32, name="ps_o", bufs=2)

        pid = nc.partition_id()
        q0 = (pid % 2) * QH

        # long-lived activation pools (stack: released near the end)
        pX1 = tc.alloc_tile_pool(name="pX1", bufs=1)
        x1 = pX1.tile([128, 4, D], F32R, name="x1")
        pAgg = tc.alloc_tile_pool(name="pAgg", bufs=1)
        agg = pAgg.tile([128, DC, QH], F32R, name="agg")

        pXC = tc.alloc_tile_pool(name="pXC", bufs=1)
        xc = pXC.tile([128, DC, L], F32R, name="xc")

        # ---------------- Phase A: dilated conv -> xc (feature-major) ---
        pA = tc.alloc_tile_pool(name="pA", bufs=1)
        xt_sb = pA.tile([128, DC, L + 4], F32R, name="xt_sb")
        nc.sync.dma_start(
            xt_sb[:], xt_d[:].rearrange("(c p) t -> p c t", p=128))
        for oj in range(4):
            paccs = [ps_t() for _ in range(4)]
            for k in range(3):
                for ic in range(DC):
                    wt = wt_tile()
                    nc.sync.dma_start(wt[:], wdc_d[k, ic, oj])
                    first = (k == 0 and ic == 0)
                    last = (k == 2 and ic == DC - 1)
                    for half in range(2):
                        for tt in range(2):
                            nc.tensor.matmul(
                                paccs[half * 2 + tt][:],
                                wt[:, half * 128:half * 128 + 128],
                                xt_sb[:, ic, tt * 512 + 2 * k:
                                      tt * 512 + 2 * k + 512],
                                start=first, stop=last)
            for half in range(2):
                oc = oj * 2 + half
                for tt in range(2):
                    nc.scalar.activation(
                        xc[:, oc, tt * 512:tt * 512 + 512],
                        paccs[half * 2 + tt][:], AF.Relu,
                        bias=bdc_sb[:, oc:oc + 1])
        if debug:
            nc.sync.dma_start(dbg["dxc"][:], xc[:])
        pA.release()

        # ---------------- Phase B: pyramid (strided convs) --------------
        pSC = tc.alloc_tile_pool(name="pSC", bufs=1)
        sc1 = pSC.tile([128, DC, LS[1]], F32R, name="sc1")
        sc2 = pSC.tile([128, DC, LS[2]], F32R, name="sc2")
        sc3 = pSC.tile([128, DC, LS[3]], F32R, name="sc3")
        pB = tc.alloc_tile_pool(name="pB", bufs=1)
        srcs = [xc, sc1, sc2]
        dsts = [sc1, sc2, sc3]
        for s in range(3):
            src, dst = srcs[s], dsts[s]
            ln = LS[s + 1]
            if s == 2:
                srcb = pB.tile([128, DC, LS[2]], BF16, name="srcb")
                nc.vector.tensor_copy(srcb[:], sc2[:])
                src = srcb
            for oj in range(4):
                paccs = [ps_t([128, ln]) for _ in range(2)]
                for k in range(CR):
                    for ic in range(DC):
                        if s == 2:
                            wt = wtb_tile()
                            nc.sync.dma_start(wt[:], wdec2_d[k, ic, oj])
                        else:
                            wt = wt_tile()
                            nc.sync.dma_start(wt[:], wdec_d[s, k, ic, oj])
                        first = (k == 0 and ic == 0)
                        last = (k == CR - 1 and ic == DC - 1)
                        for half in range(2):
                            nc.tensor.matmul(
                                paccs[half][:],
                                wt[:, half * 128:half * 128 + 128],
                                src[:, ic, k:2 * ln:2],
                                start=first, stop=last)
                for half in range(2):
                    nc.vector.tensor_copy(dst[:, oj * 2 + half, :],
                                          paccs[half][:])
        if debug:
            nc.sync.dma_start(dbg["dsc1"][:], sc1[:])
            nc.sync.dma_start(dbg["dsc3"][:], sc3[:])
        pB.release()

        # ---------------- Phase C: attention over 4 scales --------------
        pCt = tc.alloc_tile_pool(name="pCt", bufs=1)
        probsC = tc.alloc_tile_pool(name="probsC", bufs=4)
        tmpC = tc.alloc_tile_pool(name="tmpC", bufs=2)
        for s in range(S):
            src = [xc, sc1, sc2, sc3][s]
            ls = LS[s]
            kk_n = ls // 128
            nt_n = (ls + 511) // 512
            oun = pCt.tile([128, DC, QH], F32, name="oun")
            srcb16 = pCt.tile([128, DC, ls], BF16, name="srcb16")
            nc.vector.tensor_copy(srcb16[:], src[:])
            wv_tiles = None
            if debug and s == 0:
                dbg_oun_pending = True
            for g in range(4):      # head groups: heads 4g..4g+3
                oc0 = 2 * g        # d-chunks 2g, 2g+1 <-> q/k rows
                # --- queries for this group's chunks
                qs = pCt.tile([128, 2, QH], F32R, name="qs", bufs=2)
                pq = [ps_t([128, QH]) for _ in range(2)]
                for ic in range(DC):
                    wtq = wt_tile()
                    nc.sync.dma_start(wtq[:], wq_d[s, ic, g])
                    for half in range(2):
                        nc.tensor.matmul(
                            pq[half][:],
                            wtq[:, half * 128:half * 128 + 128],
                            xc[:, ic, bass.ds(q0, QH)],
                            start=(ic == 0), stop=(ic == DC - 1))
                for half in range(2):
                    nc.vector.tensor_copy(qs[:, half, :], pq[half][:])
                # --- keys (feature-major) for chunks 2g, 2g+1
                kt = pCt.tile([128, 2, ls], F32R, name="kt")
                for nt in range(nt_n):
                    n = min(512, ls - nt * 512)
                    pk = [ps_t([128, n]) for _ in range(2)]
                    for ic in range(DC):
                        wtk = wt_tile()
                        nc.sync.dma_start(wtk[:], wk_d[s, ic, g])
                        for half in range(2):
                            nc.tensor.matmul(
                                pk[half][:],
                                wtk[:, half * 128:half * 128 + 128],
                                src[:, ic, nt * 512:nt * 512 + n],
                                start=(ic == 0), stop=(ic == DC - 1))
                    for half in range(2):
                        nc.vector.tensor_copy(
                            kt[:, half, nt * 512:nt * 512 + n],
                            pk[half][:])
                # --- values (row-major, bf16, interleaved ones column)
                if g % 2 == 0:
                    wv_tiles = []
                    for ic in range(DC):
                        wt = wbig_tile(BF16)
                        nc.sync.dma_start(wt[:], wv_d[ic, g // 2])
                        wv_tiles.append(wt)
                va = pCt.tile([128, kk_n, 4 * 128], BF16, name="va",
                              bufs=2)
                nc.vector.memset(
                    va[:].rearrange("p k (h c) -> p k h c", c=128)
                    [:, :, :, 64:128], 1.0)
                for kk in range(kk_n):
                    pacc = ps_t([128, 256])
                    for ic in range(DC):
                        nc.tensor.matmul(
                            pacc[:],
                            srcb16[:, ic, kk * 128:kk * 128 + 128],
                            wv_tiles[ic][:,
                                         (g % 2) * 256:(g % 2) * 256 + 256],
                            start=(ic == 0), stop=(ic == DC - 1))
                    nc.vector.tensor_copy(
                        va[:, kk, :].rearrange("p (h c) -> p h c", c=128)
                        [:, :, 0:64],
                        pacc[:].rearrange("p (h c) -> p h c", c=64))
                # --- attention for the 4 heads
                Rg = None
                for hh in range(4):
                    half, off = hh // 2, (hh % 2) * 64
                    pov = ps_o()
                    for kk in range(kk_n):
                        pss = ps_s([128, QH])
                        nc.tensor.matmul(
                            pss[:],
                            kt[off:off + 64, half, kk * 128:kk * 128 + 128],
                            qs[off:off + 64, half, :],
                            start=True, stop=True)
                        pb = probsC.tile([128, QH], BF16, name="pb")
                        nc.scalar.activation(pb[:], pss[:], AF.Exp,
                                             bias=zcol[:],
                                             scale=1.0 / np.sqrt(DH))
                        nc.tensor.matmul(
                            pov[0:128, :],
                            va[:, kk, hh * 128:hh * 128 + 128],
                            pb[:], start=(kk == 0), stop=(kk == kk_n - 1))
                    nc.vector.tensor_copy(
                        oun[off:off + 64, oc0 + half, :], pov[0:64, :])
                    # R = w_s / den via exp(-ln(den) + ln(w_s)) on ACT
                    if hh % 2 == 0:
                        Rg = tmpC.tile([128, QH], F32, name="Rg", bufs=2)
                    lnt = tmpC.tile([64, QH], F32, name="lnt64")
                    nc.scalar.activation(lnt[:], pov[64:128, :], AF.Ln,
                                         bias=zcol[0:64, :])
                    nc.scalar.activation(Rg[off:off + 64, :], lnt[:],
                                         AF.Exp, scale=-1.0,
                                         bias=lnw_sb[0:64, s:s + 1])
                    if hh % 2 == 1:
                        oc = oc0 + half
                        if s == 0:
                            nc.vector.tensor_tensor(
                                out=agg[:, oc, :], in0=oun[:, oc, :],
                                in1=Rg[:], op=OP.mult)
                        else:
                            t = tmpC.tile([128, QH], F32, name="tagg")
                            nc.vector.tensor_tensor(
                                out=t[:], in0=oun[:, oc, :],
                                in1=Rg[:], op=OP.mult)
                            nc.vector.tensor_tensor(
                                out=agg[:, oc, :], in0=agg[:, oc, :],
                                in1=t[:], op=OP.add)
            if debug and s == 0:
                nc.gpsimd.dma_start(dbg["doun"][:], oun[:])
        if debug:
            nc.sync.dma_start(dbg["dagg"][:], agg[:])
        tmpC.release()
        probsC.release()
        pCt.release()
        pSC.release()
        pXC.release()

        # ---------------- LN machinery + late constants -----------------
        pImgs = tc.alloc_tile_pool(name="pImgs", bufs=1)
        imgs = {}
        for nm, d in [("b2", b2i_d), ("g1", g1i_d), ("t1", t1i_d),
                      ("g2", g2i_d), ("t2", t2i_d)]:
            t = pImgs.tile([128, D], F32, name=f"img_{nm}")
            nc.sync.dma_start(t[:], d[:])
            imgs[nm] = t
        b1_sb = pImgs.tile([128, FC], F32, name="b1_sb")
        nc.sync.dma_start(b1_sb[:], b1_d[:])
        pLN = tc.alloc_tile_pool(name="pLN", bufs=2)
        tmpLN = tc.alloc_tile_pool(name="tmpLN", bufs=3)

        def layer_norm(xin, xout, g_img, t_img):
            # xin/xout: [128, D] APs; row-wise LN along free dim
            musum = tmpLN.tile([128, 1], F32, name="lnmu")
            nc.vector.reduce_sum(musum[:], xin, axis=mybir.AxisListType.X)
            negmu = tmpLN.tile([128, 1], F32, name="lnnm")
            nc.scalar.mul(negmu[:], musum[:], -1.0 / D)
            xm = pLN.tile([128, D], F32, name="lnxm")
            nc.scalar.activation(xm[:], xin, AF.Identity, bias=negmu[:])
            sq = pLN.tile([128, D], F32, name="lnsq")
            ssq = tmpLN.tile([128, 1], F32, name="lnss")
            nc.scalar.activation(sq[:], xm[:], AF.Square, bias=zcol[:],
                                 accum_out=ssq[:])
            std = tmpLN.tile([128, 1], F32, name="lnsd")
            nc.scalar.activation(std[:], ssq[:], AF.Sqrt, scale=1.0 / D,
                                 bias=epscol[:])
            rstd = tmpLN.tile([128, 1], F32, name="lnrs")
            nc.vector.reciprocal(rstd[:], std[:])
            t = pLN.tile([128, D], F32, name="lnt")
            nc.vector.scalar_tensor_tensor(
                out=t[:], in0=xm[:], scalar=rstd[:, 0:1], in1=g_img[:],
                op0=OP.mult, op1=OP.mult)
            nc.vector.tensor_tensor(out=xout, in0=t[:], in1=t_img[:],
                                    op=OP.add)

        # ---------------- Phase D: Wo + residual + LN1 ------------------
        pD = tc.alloc_tile_pool(name="pD", bufs=1)
        xres_sb = pD.tile([128, 4, D], F32, name="xres_sb")
        nc.sync.dma_start(
            xres_sb[:], xres_d[:].rearrange("(c p) d -> p c d", p=128))
        xpms = [pD.tile([128, D], F32, name=f"xpm{i}") for i in range(4)]
        for os_ in range(2):
            pms = [ps_t() for _ in range(4)]
            for dc in range(DC):
                wt = wbig_tile()
                nc.sync.dma_start(wt[:], wo_d[dc, os_])
                for qc in range(4):
                    nc.tensor.matmul(
                        pms[qc][:], agg[:, dc, qc * 128:qc * 128 + 128],
                        wt[:], start=(dc == 0), stop=(dc == DC - 1))
            for qc in range(4):
                nc.vector.tensor_tensor(
                    out=xpms[qc][:, os_ * 512:os_ * 512 + 512],
                    in0=pms[qc][:],
                    in1=xres_sb[:, qc, os_ * 512:os_ * 512 + 512],
                    op=OP.add)
        for qc in range(4):
            layer_norm(xpms[qc][:], x1[:, qc, :], imgs["g1"], imgs["t1"])
        if debug:
            nc.sync.dma_start(dbg["dx1"][:], x1[:])
        pD.release()

        # ---------------- Phase E+F: transpose, FFN ---------------------
        pEF = tc.alloc_tile_pool(name="pEF", bufs=1)
        x1t = pEF.tile([128, DC, QH], BF16, name="x1t")
        for qc in range(4):
            for dc in range(DC):
                pt = ps_s([128, 128], F32R)
                nc.tensor.transpose(pt[:], x1[:, qc, dc * 128:dc * 128 + 128],
                                    ident[:])
                nc.vector.tensor_copy(x1t[:, dc, qc * 128:qc * 128 + 128],
                                      pt[:])

        hT = pEF.tile([128, FC, QH], BF16, name="hT")
        for fj in range(16):
            paccs = [ps_t([128, QH]) for _ in range(2)]
            for ic in range(DC):
                wt = wtb_tile()
                nc.sync.dma_start(wt[:], w1_d[ic, fj])
                for half in range(2):
                    nc.tensor.matmul(
                        paccs[half][:], wt[:, half * 128:half * 128 + 128],
                        x1t[:, ic, :], start=(ic == 0), stop=(ic == DC - 1))
            for half in range(2):
                fc = fj * 2 + half
                nc.scalar.activation(hT[:, fc, :], paccs[half][:], AF.Gelu,
                                     bias=b1_sb[:, fc:fc + 1])

        pF2 = tc.alloc_tile_pool(name="pF2", bufs=1)
        tfF = tc.alloc_tile_pool(name="tfF", bufs=2)
        x2s = [pF2.tile([128, D], F32, name=f"x2_{i}") for i in range(4)]
        for os_ in range(2):
            pfs = [ps_t() for _ in range(4)]
            for fc in range(FC):
                wt = wbig_tile(BF16)
                nc.sync.dma_start(wt[:], w2_d[fc, os_])
                for qc in range(4):
                    nc.tensor.matmul(
                        pfs[qc][:], hT[:, fc, qc * 128:qc * 128 + 128],
                        wt[:], start=(fc == 0), stop=(fc == FC - 1))
            for qc in range(4):
                t = tfF.tile([128, 512], F32, name="tf")
                nc.vector.tensor_tensor(
                    out=t[:], in0=pfs[qc][:],
                    in1=imgs["b2"][:, os_ * 512:os_ * 512 + 512], op=OP.add)
                nc.vector.tensor_tensor(
                    out=x2s[qc][:, os_ * 512:os_ * 512 + 512], in0=t[:],
                    in1=x1[:, qc, os_ * 512:os_ * 512 + 512], op=OP.add)
        for qc in range(4):
            outq = tfF.tile([128, D], F32, name="outq")
            layer_norm(x2s[qc][:], outq[:], imgs["g2"], imgs["t2"])
            nc.sync.dma_start(
                out_d[:].rearrange("(c p) d -> p c d", p=128)[:, qc, :],
                outq[:])

        for p in (tfF, pF2, pEF, tmpLN, pLN, pImgs, pAgg, pX1,
                  pspool, wpool, const, dram):
            p.release()
    nc.compile()
    return nc


def _chunk_lhst(w, dt=np.float32):
    # [I, O] -> [I/128, O/256, 128, 256] contiguous blocks
    i, o = w.shape
    return np.ascontiguousarray(
        w.reshape(i // 128, 128, o // 256, 256).transpose(0, 2, 1, 3)
    ).astype(dt, copy=False)


def _chunk_rhs(w):
    # [I, O] -> [I/128, O/512, 128, 512]
    i, o = w.shape
    return np.ascontiguousarray(
        w.reshape(i // 128, 128, o // 512, 512).transpose(0, 2, 1, 3))


def _chunk_rhs_bf16(w):
    return _chunk_rhs(w).astype(ml_dtypes.bfloat16)


def _prep(inputs):
    f = {k: np.asarray(v, dtype=np.float32) for k, v in inputs.items()}
    W_dc, W_dec = f["W_dc"], f["W_dec"]
    wdc = np.stack([_chunk_lhst(W_dc[:, :, k].T) for k in range(3)])
    wdec = np.stack([
        np.stack([_chunk_lhst(W_dec[s, :, :, k].T) for k in range(CR)])
        for s in range(2)])
    wdec2 = np.stack([
        _chunk_lhst(W_dec[2, :, :, k].T, ml_dtypes.bfloat16)
        for k in range(CR)])
    wq = np.stack([_chunk_lhst(f["Wq"][s]) for s in range(S)])
    wk = np.stack([_chunk_lhst(f["Wk"][s]) for s in range(S)])

    ex = np.exp(f["agg_logits"] - f["agg_logits"].max())
    w = (ex / ex.sum()).astype(np.float32)
    aux = np.float32(-(w * np.log(w + np.float32(1e-9))).sum())

    shared = {
        "wdc": wdc,
        "bdc": np.ascontiguousarray(f["b_dc"].reshape(DC, 128).T),
        "wdec": wdec, "wdec2": wdec2, "wq": wq, "wk": wk,
        "wv": _chunk_rhs_bf16(f["Wv"]), "wo": _chunk_rhs(f["Wo"]),
        "w1": _chunk_lhst(f["W1"], ml_dtypes.bfloat16),
        "b1": np.ascontiguousarray(f["b1"].reshape(FC, 128).T),
        "w2": _chunk_rhs_bf16(f["W2"]),
        "b2img": np.ascontiguousarray(
            np.broadcast_to(f["b2"], (128, D))),
        "g1img": np.ascontiguousarray(np.broadcast_to(f["gamma1"], (128, D))),
        "bt1img": np.ascontiguousarray(np.broadcast_to(f["beta1"], (128, D))),
        "g2img": np.ascontiguousarray(np.broadcast_to(f["gamma2"], (128, D))),
        "bt2img": np.ascontiguousarray(np.broadcast_to(f["beta2"], (128, D))),
        "wvec": np.ascontiguousarray(np.broadcast_to(w, (H, S))),
        "lnwimg": np.ascontiguousarray(
            np.broadcast_to(np.log(w), (128, S)).astype(np.float32)),
        "ident": np.eye(128, dtype=np.float32),
    }
    in_maps = []
    x = f["x"]
    for c in range(NC):
        b, half = c // 2, c % 2
        xt = np.zeros((D, L + 4), np.float32)
        xt[:, 2:L + 2] = x[b].T
        m = dict(shared)
        m["xt_pad"] = xt
        m["xres"] = np.ascontiguousarray(x[b, half * QH:(half + 1) * QH, :])
        in_maps.append(m)
    return in_maps, aux


def run(inputs, trace=False, debug=False):
    key = ("ncd" if debug else "nc")
    if key not in _CACHE:
        _CACHE[key] = _build(debug=debug)
    nc = _CACHE[key]
    in_maps, aux = _prep(inputs)
    res = run_bass_kernel_spmd(nc, in_maps, core_ids=list(range(NC)),
                               trace=trace)
    out = np.empty((B, L, D), np.float32)
    for c in range(NC):
        b, half = c // 2, c % 2
        out[b, half * QH:(half + 1) * QH, :] = res.results[c]["out"]
    return (out, aux), res


def kernel(**inputs):
    (out, aux), _ = run(inputs, trace=False)
    return out, aux


# revision 19
# speedup vs baseline: 1.1230x; 1.0423x over previous
"""Trainium2 Bass kernel for nn_MAHABlock (multiscale-attention block).

Sharding: 8 cores = 4 batches x 2 query-halves. Each core redundantly
computes the conv front-end + key/value pyramid for its batch (needed in
full by every query), and computes queries / attention / FFN only for its
512-row half. No cross-core communication.

On-chip layout: activations are kept feature-major ([d, t]) so weight
blocks serve as the stationary matmul operand, except where row-major is
required (v for attention, LN row statistics). All large matmuls run as
float32r (TF32-like, full rate at N>=512); attention probabilities and a
few tiny side matmuls use bf16.
"""

import numpy as np
import ml_dtypes

import concourse.bass as bass
import concourse.mybir as mybir
import concourse.tile as tile
from concourse import bacc
from concourse.bass_utils import run_bass_kernel_spmd

F32 = mybir.dt.float32
F32R = mybir.dt.float32r
BF16 = mybir.dt.bfloat16
AF = mybir.ActivationFunctionType
OP = mybir.AluOpType

B, L, D, H, F, S, CR = 4, 1024, 1024, 16, 4096, 4, 2
DH = D // H          # 64
QH = L // 2          # 512 query rows per core
DC = D // 128        # 8
FC = F // 128        # 32
NC = 8
LS = [L >> s for s in range(S)]   # [1024, 512, 256, 128]
EPS = 1e-5

_CACHE = {}


def _build(debug=False):
    nc = bacc.Bacc(None, target_bir_lowering=False)
    with tile.TileContext(nc) as tc:
        dram = tc.alloc_tile_pool(name="dram", bufs=1, space="DRAM")

        def din(name, shape, dt=F32R):
            return dram.tile(shape, dt, kind="ExternalInput", name=name,
                             uniquify=False)

        xt_d = din("xt_pad", [D, L + 4], BF16)
        xres_d = din("xres", [QH, D], F32)
        wdc_d = din("wdc", [3, DC, 4, 128, 256], BF16)
        bdc_d = din("bdc", [128, DC], F32)
        wdec_d = din("wdec", [3, CR, DC, 4, 128, 256], BF16)
        wq_d = din("wq", [S, DC, 4, 128, 256], BF16)
        wk_d = din("wk", [S, DC, 4, 128, 256], BF16)
        wv_d = din("wv", [DC, 2, 128, 512], BF16)
        wo_d = din("wo", [DC, 2, 128, 512])
        w1_d = din("w1", [DC, 16, 128, 256], BF16)
        b1_d = din("b1", [128, FC], F32)
        w2_d = din("w2", [FC, 2, 128, 512], BF16)
        b2i_d = din("b2img", [128, D], F32)
        g1i_d = din("g1img", [128, D], F32)
        t1i_d = din("bt1img", [128, D], F32)
        g2i_d = din("g2img", [128, D], F32)
        t2i_d = din("bt2img", [128, D], F32)
        wvec_d = din("wvec", [H, S], F32)
        lnw_d = din("lnwimg", [128, S], F32)
        id_d = din("ident", [128, 128])
        out_d = dram.tile([QH, D], F32, kind="ExternalOutput", name="out",
                          uniquify=False)
        if debug:
            dbg = {}
            for nm, shp in [("dxc", [128, DC, L]), ("dsc1", [128, DC, 512]),
                            ("dsc3", [128, DC, 128]),
                            ("dagg", [128, DC, QH]), ("dx1", [128, 4, D]),
                            ("doun", [128, DC, QH])]:
                dbg[nm] = dram.tile(shp, F32R, kind="ExternalOutput",
                                    name=nm, uniquify=False)

        # ---- constants (live for the whole kernel) ----
        const = tc.alloc_tile_pool(name="const", bufs=1)
        bdc_sb = const.tile([128, DC], F32, name="bdc_sb")
        nc.sync.dma_start(bdc_sb[:], bdc_d[:])
        lnw_sb = const.tile([128, S], F32, name="lnw_sb")
        nc.sync.dma_start(lnw_sb[:], lnw_d[:])
        ident = const.tile([128, 128], F32R, name="ident")
        nc.sync.dma_start(ident[:], id_d[:])
        zcol = const.tile([128, 1], F32, name="zcol")
        nc.vector.memset(zcol[:], 0.0)
        epscol = const.tile([128, 1], F32, name="epscol")
        nc.vector.memset(epscol[:], EPS)

        # global weight-streaming + psum pools (whole kernel)
        wpool = tc.alloc_tile_pool(name="wpool", bufs=8)
        pspool = tc.alloc_tile_pool(name="pspool", bufs=4, space="PSUM")

        def wt_tile():
            return wpool.tile([128, 256], F32R, name="wt", bufs=8)

        def wtb_tile():
            return wpool.tile([128, 256], BF16, name="wtb", bufs=4)

        def wbig_tile(dt=F32R):
            return wpool.tile([128, 512], dt, name="wbig", bufs=8)

        def ps_t(shape=None):
            return pspool.tile(shape or [128, 512], F32, name="ps_t",
                               bufs=4)

        def ps_s(shape=None, dt=F32):
            return pspool.tile(shape or [128, 512], dt, name="ps_s",
                               bufs=2)

        def ps_o():
            return pspool.tile([128, QH], F32, name="ps_o", bufs=2)

        pid = nc.partition_id()
        q0 = (pid % 2) * QH

        # long-lived activation pools (stack: released near the end)
        pX1 = tc.alloc_tile_pool(name="pX1", bufs=1)
        x1 = pX1.tile([128, 4, D], F32R, name="x1")
        pAgg = tc.alloc_tile_pool(name="pAgg", bufs=1)
        agg = pAgg.tile([128, DC, QH], F32R, name="agg")

        pXC = tc.alloc_tile_pool(name="pXC", bufs=1)
        xc = pXC.tile([128, DC, L], BF16, name="xc")

        # ---------------- Phase A: dilated conv -> xc (feature-major) ---
        pA = tc.alloc_tile_pool(name="pA", bufs=1)
        xt_sb = pA.tile([128, DC, L + 4], BF16, name="xt_sb")
        nc.sync.dma_start(
            xt_sb[:], xt_d[:].rearrange("(c p) t -> p c t", p=128))
        for oj in range(4):
            paccs = [ps_t() for _ in range(4)]
            for k in range(3):
                for ic in range(DC):
                    wt = wtb_tile()
                    nc.sync.dma_start(wt[:], wdc_d[k, ic, oj])
                    first = (k == 0 and ic == 0)
                    last = (k == 2 and ic == DC - 1)
                    for half in range(2):
                        for tt in range(2):
                            nc.tensor.matmul(
                                paccs[half * 2 + tt][:],
                                wt[:, half * 128:half * 128 + 128],
                                xt_sb[:, ic, tt * 512 + 2 * k:
                                      tt * 512 + 2 * k + 512],
                                start=first, stop=last)
            for half in range(2):
                oc = oj * 2 + half
                for tt in range(2):
                    nc.scalar.activation(
                        xc[:, oc, tt * 512:tt * 512 + 512],
                        paccs[half * 2 + tt][:], AF.Relu,
                        bias=bdc_sb[:, oc:oc + 1])
        if debug:
            nc.sync.dma_start(dbg["dxc"][:], xc[:])
        pA.release()

        # ---------------- Phase B: pyramid (strided convs) --------------
        pSC = tc.alloc_tile_pool(name="pSC", bufs=1)
        sc1 = pSC.tile([128, DC, LS[1]], BF16, name="sc1")
        sc2 = pSC.tile([128, DC, LS[2]], BF16, name="sc2")
        sc3 = pSC.tile([128, DC, LS[3]], BF16, name="sc3")
        pB = tc.alloc_tile_pool(name="pB", bufs=1)
        srcs = [xc, sc1, sc2]
        dsts = [sc1, sc2, sc3]
        for s in range(3):
            src, dst = srcs[s], dsts[s]
            ln = LS[s + 1]
            for oj in range(4):
                paccs = [ps_t([128, ln]) for _ in range(2)]
                for k in range(CR):
                    for ic in range(DC):
                        wt = wtb_tile()
                        nc.sync.dma_start(wt[:], wdec_d[s, k, ic, oj])
                        first = (k == 0 and ic == 0)
                        last = (k == CR - 1 and ic == DC - 1)
                        for half in range(2):
                            nc.tensor.matmul(
                                paccs[half][:],
                                wt[:, half * 128:half * 128 + 128],
                                src[:, ic, k:2 * ln:2],
                                start=first, stop=last)
                for half in range(2):
                    nc.vector.tensor_copy(dst[:, oj * 2 + half, :],
                                          paccs[half][:])
        if debug:
            nc.sync.dma_start(dbg["dsc1"][:], sc1[:])
            nc.sync.dma_start(dbg["dsc3"][:], sc3[:])
        pB.release()

        # ---------------- Phase C: attention over 4 scales --------------
        pCt = tc.alloc_tile_pool(name="pCt", bufs=1)
        probsC = tc.alloc_tile_pool(name="probsC", bufs=4)
        tmpC = tc.alloc_tile_pool(name="tmpC", bufs=2)
        for s in range(S):
            src = [xc, sc1, sc2, sc3][s]
            ls = LS[s]
            kk_n = ls // 128
            nt_n = (ls + 511) // 512
            oun = pCt.tile([128, DC, QH], F32, name="oun")
            wv_tiles = None
            if debug and s == 0:
                dbg_oun_pending = True
            for g in range(4):      # head groups: heads 4g..4g+3
                oc0 = 2 * g        # d-chunks 2g, 2g+1 <-> q/k rows
                # --- queries for this group's chunks
                qs = pCt.tile([128, 2, QH], BF16, name="qs", bufs=2)
                pq = [ps_t([128, QH]) for _ in range(2)]
                for ic in range(DC):
                    wtq = wtb_tile()
                    nc.sync.dma_start(wtq[:], wq_d[s, ic, g])
                    for half in range(2):
                        nc.tensor.matmul(
                            pq[half][:],
                            wtq[:, half * 128:half * 128 + 128],
                            xc[:, ic, bass.ds(q0, QH)],
                            start=(ic == 0), stop=(ic == DC - 1))
                for half in range(2):
                    nc.vector.tensor_copy(qs[:, half, :], pq[half][:])
                # --- keys (feature-major) for chunks 2g, 2g+1
                kt = pCt.tile([128, 2, ls], BF16, name="kt")
                for nt in range(nt_n):
                    n = min(512, ls - nt * 512)
                    pk = [ps_t([128, n]) for _ in range(2)]
                    for ic in range(DC):
                        wtk = wtb_tile()
                        nc.sync.dma_start(wtk[:], wk_d[s, ic, g])
                        for half in range(2):
                            nc.tensor.matmul(
                                pk[half][:],
                                wtk[:, half * 128:half * 128 + 128],
                                src[:, ic, nt * 512:nt * 512 + n],
                                start=(ic == 0), stop=(ic == DC - 1))
                    for half in range(2):
                        nc.vector.tensor_copy(
                            kt[:, half, nt * 512:nt * 512 + n],
                            pk[half][:])
                # --- values (row-major, bf16, interleaved ones column)
                if g % 2 == 0:
                    wv_tiles = []
                    for ic in range(DC):
                        wt = wbig_tile(BF16)
                        nc.sync.dma_start(wt[:], wv_d[ic, g // 2])
                        wv_tiles.append(wt)
                va = pCt.tile([128, kk_n, 4 * 128], BF16, name="va",
                              bufs=2)
                nc.vector.memset(
                    va[:].rearrange("p k (h c) -> p k h c", c=128)
                    [:, :, :, 64:128], 1.0)
                for kk in range(kk_n):
                    pacc = ps_t([128, 256])
                    for ic in range(DC):
                        nc.tensor.matmul(
                            pacc[:],
                            src[:, ic, kk * 128:kk * 128 + 128],
                            wv_tiles[ic][:,
                                         (g % 2) * 256:(g % 2) * 256 + 256],
                            start=(ic == 0), stop=(ic == DC - 1))
                    nc.vector.tensor_copy(
                        va[:, kk, :].rearrange("p (h c) -> p h c", c=128)
                        [:, :, 0:64],
                        pacc[:].rearrange("p (h c) -> p h c", c=64))
                # --- attention for the 4 heads
                Rg = None
                for hh in range(4):
                    half, off = hh // 2, (hh % 2) * 64
                    pov = ps_o()
                    for kk in range(kk_n):
                        pss = ps_s([128, QH])
                        nc.tensor.matmul(
                            pss[:],
                            kt[off:off + 64, half, kk * 128:kk * 128 + 128],
                            qs[off:off + 64, half, :],
                            start=True, stop=True)
                        pb = probsC.tile([128, QH], BF16, name="pb")
                        nc.scalar.activation(pb[:], pss[:], AF.Exp,
                                             bias=zcol[:],
                                             scale=1.0 / np.sqrt(DH))
                        nc.tensor.matmul(
                            pov[0:128, :],
                            va[:, kk, hh * 128:hh * 128 + 128],
                            pb[:], start=(kk == 0), stop=(kk == kk_n - 1))
                    nc.vector.tensor_copy(
                        oun[off:off + 64, oc0 + half, :], pov[0:64, :])
                    # R = w_s / den via exp(-ln(den) + ln(w_s)) on ACT
                    if hh % 2 == 0:
                        Rg = tmpC.tile([128, QH], F32, name="Rg", bufs=2)
                    lnt = tmpC.tile([64, QH], F32, name="lnt64")
                    nc.scalar.activation(lnt[:], pov[64:128, :], AF.Ln,
                                         bias=zcol[0:64, :])
                    nc.scalar.activation(Rg[off:off + 64, :], lnt[:],
                                         AF.Exp, scale=-1.0,
                                         bias=lnw_sb[0:64, s:s + 1])
                    if hh % 2 == 1:
                        oc = oc0 + half
                        if s == 0:
                            nc.vector.tensor_tensor(
                                out=agg[:, oc, :], in0=oun[:, oc, :],
                                in1=Rg[:], op=OP.mult)
                        else:
                            t = tmpC.tile([128, QH], F32, name="tagg")
                            nc.vector.tensor_tensor(
                                out=t[:], in0=oun[:, oc, :],
                                in1=Rg[:], op=OP.mult)
                            nc.vector.tensor_tensor(
                                out=agg[:, oc, :], in0=agg[:, oc, :],
                                in1=t[:], op=OP.add)
            if debug and s == 0:
                nc.gpsimd.dma_start(dbg["doun"][:], oun[:])
        if debug:
            nc.sync.dma_start(dbg["dagg"][:], agg[:])
        tmpC.release()
        probsC.release()
        pCt.release()
        pSC.release()
        pXC.release()

        # ---------------- LN machinery + late constants -----------------
        pImgs = tc.alloc_tile_pool(name="pImgs", bufs=1)
        imgs = {}
        for nm, d in [("b2", b2i_d), ("g1", g1i_d), ("t1", t1i_d),
                      ("g2", g2i_d), ("t2", t2i_d)]:
            t = pImgs.tile([128, D], F32, name=f"img_{nm}")
            nc.sync.dma_start(t[:], d[:])
            imgs[nm] = t
        b1_sb = pImgs.tile([128, FC], F32, name="b1_sb")
        nc.sync.dma_start(b1_sb[:], b1_d[:])
        pLN = tc.alloc_tile_pool(name="pLN", bufs=2)
        tmpLN = tc.alloc_tile_pool(name="tmpLN", bufs=3)

        def layer_norm(xin, xout, g_img, t_img):
            # xin/xout: [128, D] APs; row-wise LN along free dim
            musum = tmpLN.tile([128, 1], F32, name="lnmu")
            nc.vector.reduce_sum(musum[:], xin, axis=mybir.AxisListType.X)
            negmu = tmpLN.tile([128, 1], F32, name="lnnm")
            nc.scalar.mul(negmu[:], musum[:], -1.0 / D)
            xm = pLN.tile([128, D], F32, name="lnxm")
            nc.scalar.activation(xm[:], xin, AF.Identity, bias=negmu[:])
            sq = pLN.tile([128, D], F32, name="lnsq")
            ssq = tmpLN.tile([128, 1], F32, name="lnss")
            nc.scalar.activation(sq[:], xm[:], AF.Square, bias=zcol[:],
                                 accum_out=ssq[:])
            std = tmpLN.tile([128, 1], F32, name="lnsd")
            nc.scalar.activation(std[:], ssq[:], AF.Sqrt, scale=1.0 / D,
                                 bias=epscol[:])
            rstd = tmpLN.tile([128, 1], F32, name="lnrs")
            nc.vector.reciprocal(rstd[:], std[:])
            t = pLN.tile([128, D], F32, name="lnt")
            nc.vector.scalar_tensor_tensor(
                out=t[:], in0=xm[:], scalar=rstd[:, 0:1], in1=g_img[:],
                op0=OP.mult, op1=OP.mult)
            nc.vector.tensor_tensor(out=xout, in0=t[:], in1=t_img[:],
                                    op=OP.add)

        # ---------------- Phase D: Wo + residual + LN1 ------------------
        pD = tc.alloc_tile_pool(name="pD", bufs=1)
        xres_sb = pD.tile([128, 4, D], F32, name="xres_sb")
        nc.sync.dma_start(
            xres_sb[:], xres_d[:].rearrange("(c p) d -> p c d", p=128))
        xpms = [pD.tile([128, D], F32, name=f"xpm{i}") for i in range(4)]
        for os_ in range(2):
            pms = [ps_t() for _ in range(4)]
            for dc in range(DC):
                wt = wbig_tile()
                nc.sync.dma_start(wt[:], wo_d[dc, os_])
                for qc in range(4):
                    nc.tensor.matmul(
                        pms[qc][:], agg[:, dc, qc * 128:qc * 128 + 128],
                        wt[:], start=(dc == 0), stop=(dc == DC - 1))
            for qc in range(4):
                nc.vector.tensor_tensor(
                    out=xpms[qc][:, os_ * 512:os_ * 512 + 512],
                    in0=pms[qc][:],
                    in1=xres_sb[:, qc, os_ * 512:os_ * 512 + 512],
                    op=OP.add)
        for qc in range(4):
            layer_norm(xpms[qc][:], x1[:, qc, :], imgs["g1"], imgs["t1"])
        if debug:
            nc.sync.dma_start(dbg["dx1"][:], x1[:])
        pD.release()

        # ---------------- Phase E+F: transpose, FFN ---------------------
        pEF = tc.alloc_tile_pool(name="pEF", bufs=1)
        x1t = pEF.tile([128, DC, QH], BF16, name="x1t")
        for qc in range(4):
            for dc in range(DC):
                pt = ps_s([128, 128], F32R)
                nc.tensor.transpose(pt[:], x1[:, qc, dc * 128:dc * 128 + 128],
                                    ident[:])
                nc.vector.tensor_copy(x1t[:, dc, qc * 128:qc * 128 + 128],
                                      pt[:])

        hT = pEF.tile([128, FC, QH], BF16, name="hT")
        for fj in range(16):
            paccs = [ps_t([128, QH]) for _ in range(2)]
            for ic in range(DC):
                wt = wtb_tile()
                nc.sync.dma_start(wt[:], w1_d[ic, fj])
                for half in range(2):
                    nc.tensor.matmul(
                        paccs[half][:], wt[:, half * 128:half * 128 + 128],
                        x1t[:, ic, :], start=(ic == 0), stop=(ic == DC - 1))
            for half in range(2):
                fc = fj * 2 + half
                nc.scalar.activation(hT[:, fc, :], paccs[half][:], AF.Gelu,
                                     bias=b1_sb[:, fc:fc + 1])

        pF2 = tc.alloc_tile_pool(name="pF2", bufs=1)
        tfF = tc.alloc_tile_pool(name="tfF", bufs=2)
        x2s = [pF2.tile([128, D], F32, name=f"x2_{i}") for i in range(4)]
        for os_ in range(2):
            pfs = [ps_t() for _ in range(4)]
            for fc in range(FC):
                wt = wbig_tile(BF16)
                nc.sync.dma_start(wt[:], w2_d[fc, os_])
                for qc in range(4):
                    nc.tensor.matmul(
                        pfs[qc][:], hT[:, fc, qc * 128:qc * 128 + 128],
                        wt[:], start=(fc == 0), stop=(fc == FC - 1))
            for qc in range(4):
                t = tfF.tile([128, 512], F32, name="tf")
                nc.vector.tensor_tensor(
                    out=t[:], in0=pfs[qc][:],
                    in1=imgs["b2"][:, os_ * 512:os_ * 512 + 512], op=OP.add)
                nc.vector.tensor_tensor(
                    out=x2s[qc][:, os_ * 512:os_ * 512 + 512], in0=t[:],
                    in1=x1[:, qc, os_ * 512:os_ * 512 + 512], op=OP.add)
        for qc in range(4):
            outq = tfF.tile([128, D], F32, name="outq")
            layer_norm(x2s[qc][:], outq[:], imgs["g2"], imgs["t2"])
            nc.sync.dma_start(
                out_d[:].rearrange("(c p) d -> p c d", p=128)[:, qc, :],
                outq[:])

        for p in (tfF, pF2, pEF, tmpLN, pLN, pImgs, pAgg, pX1,
                  pspool, wpool, const, dram):
            p.release()
    nc.compile()
    return nc


def _chunk_lhst(w, dt=np.float32):
    # [I, O] -> [I/128, O/256, 128, 256] contiguous blocks
    i, o = w.shape
    return np.ascontiguousarray(
        w.reshape(i // 128, 128, o // 256, 256).transpose(0, 2, 1, 3)
    ).astype(dt, copy=False)


def _chunk_rhs(w):
    # [I, O] -> [I/128, O/512, 128, 512]
    i, o = w.shape
    return np.ascontiguousarray(
        w.reshape(i // 128, 128, o // 512, 512).transpose(0, 2, 1, 3))


def _chunk_rhs_bf16(w):
    return _chunk_rhs(w).astype(ml_dtypes.bfloat16)


def _prep(inputs):
    f = {k: np.asarray(v, dtype=np.float32) for k, v in inputs.items()}
    W_dc, W_dec = f["W_dc"], f["W_dec"]
    bf = ml_dtypes.bfloat16
    wdc = np.stack([_chunk_lhst(W_dc[:, :, k].T, bf) for k in range(3)])
    wdec = np.stack([
        np.stack([_chunk_lhst(W_dec[s, :, :, k].T, bf) for k in range(CR)])
        for s in range(3)])
    wq = np.stack([_chunk_lhst(f["Wq"][s], bf) for s in range(S)])
    wk = np.stack([_chunk_lhst(f["Wk"][s], bf) for s in range(S)])

    ex = np.exp(f["agg_logits"] - f["agg_logits"].max())
    w = (ex / ex.sum()).astype(np.float32)
    aux = np.float32(-(w * np.log(w + np.float32(1e-9))).sum())

    shared = {
        "wdc": wdc,
        "bdc": np.ascontiguousarray(f["b_dc"].reshape(DC, 128).T),
        "wdec": wdec, "wq": wq, "wk": wk,
        "wv": _chunk_rhs_bf16(f["Wv"]), "wo": _chunk_rhs(f["Wo"]),
        "w1": _chunk_lhst(f["W1"], ml_dtypes.bfloat16),
        "b1": np.ascontiguousarray(f["b1"].reshape(FC, 128).T),
        "w2": _chunk_rhs_bf16(f["W2"]),
        "b2img": np.ascontiguousarray(
            np.broadcast_to(f["b2"], (128, D))),
        "g1img": np.ascontiguousarray(np.broadcast_to(f["gamma1"], (128, D))),
        "bt1img": np.ascontiguousarray(np.broadcast_to(f["beta1"], (128, D))),
        "g2img": np.ascontiguousarray(np.broadcast_to(f["gamma2"], (128, D))),
        "bt2img": np.ascontiguousarray(np.broadcast_to(f["beta2"], (128, D))),
        "wvec": np.ascontiguousarray(np.broadcast_to(w, (H, S))),
        "lnwimg": np.ascontiguousarray(
            np.broadcast_to(np.log(w), (128, S)).astype(np.float32)),
        "ident": np.eye(128, dtype=np.float32),
    }
    in_maps = []
    x = f["x"]
    for c in range(NC):
        b, half = c // 2, c % 2
        xt = np.zeros((D, L + 4), np.float32)
        xt[:, 2:L + 2] = x[b].T
        m = dict(shared)
        m["xt_pad"] = xt.astype(ml_dtypes.bfloat16)
        m["xres"] = np.ascontiguousarray(x[b, half * QH:(half + 1) * QH, :])
        in_maps.append(m)
    return in_maps, aux


def run(inputs, trace=False, debug=False):
    key = ("ncd" if debug else "nc")
    if key not in _CACHE:
        _CACHE[key] = _build(debug=debug)
    nc = _CACHE[key]
    in_maps, aux = _prep(inputs)
    res = run_bass_kernel_spmd(nc, in_maps, core_ids=list(range(NC)),
                               trace=trace)
    out = np.empty((B, L, D), np.float32)
    for c in range(NC):
        b, half = c // 2, c % 2
        out[b, half * QH:(half + 1) * QH, :] = res.results[c]["out"]
    return (out, aux), res


def kernel(**inputs):
    (out, aux), _ = run(inputs, trace=False)
    return out, aux


# revision 23
# speedup vs baseline: 1.3048x; 1.1619x over previous
"""Trainium2 Bass kernel for nn_MAHABlock (multiscale-attention block).

Sharding: 8 cores = 4 batches x 2 query-halves. Each core redundantly
computes the conv front-end + key/value pyramid for its batch (needed in
full by every query), and computes queries / attention / FFN only for its
512-row half. No cross-core communication.

On-chip layout: activations are kept feature-major ([d, t]) so weight
blocks serve as the stationary matmul operand, except where row-major is
required (v for attention, LN row statistics). All large matmuls run as
float32r (TF32-like, full rate at N>=512); attention probabilities and a
few tiny side matmuls use bf16.
"""

import numpy as np
import ml_dtypes

import concourse.bass as bass
import concourse.mybir as mybir
import concourse.tile as tile
from concourse import bacc
from concourse import bass_utils
from concourse.bass_utils import run_bass_kernel_spmd

_orig_run_command = bass_utils.run_command


def _patched_run_command(cmd, *a, **kw):
    return _orig_run_command(cmd, *a, **kw)


bass_utils.run_command = _patched_run_command

F32 = mybir.dt.float32
F32R = mybir.dt.float32r
BF16 = mybir.dt.bfloat16
AF = mybir.ActivationFunctionType
OP = mybir.AluOpType

B, L, D, H, F, S, CR = 4, 1024, 1024, 16, 4096, 4, 2
DH = D // H          # 64
QH = L // 2          # 512 query rows per core
DC = D // 128        # 8
FC = F // 128        # 32
NC = 8
LS = [L >> s for s in range(S)]   # [1024, 512, 256, 128]
EPS = 1e-5

_CACHE = {}


def _build(debug=False):
    nc = bacc.Bacc(None, target_bir_lowering=False)
    with tile.TileContext(nc) as tc:
        dram = tc.alloc_tile_pool(name="dram", bufs=1, space="DRAM")

        def din(name, shape, dt=F32R):
            return dram.tile(shape, dt, kind="ExternalInput", name=name,
                             uniquify=False)

        xt_d = din("xt_pad", [D, L + 4], BF16)
        xres_d = din("xres", [QH, D], F32)
        wdc_d = din("wdc", [3, DC, 4, 128, 256], BF16)
        bdc_d = din("bdc", [128, DC], F32)
        wdec_d = din("wdec", [3, CR, DC, 4, 128, 256], BF16)
        wq_d = din("wq", [S, DC, 4, 128, 256], BF16)
        wk_d = din("wk", [S, DC, 4, 128, 256], BF16)
        wv_d = din("wv", [DC, 2, 128, 512], BF16)
        wo_d = din("wo", [DC, 2, 128, 512], BF16)
        w1_d = din("w1", [DC, 16, 128, 256], BF16)
        b1_d = din("b1", [128, FC], F32)
        w2_d = din("w2", [FC, 2, 128, 512], BF16)
        b2i_d = din("b2img", [128, D], F32)
        g1i_d = din("g1img", [128, D], F32)
        t1i_d = din("bt1img", [128, D], F32)
        g2i_d = din("g2img", [128, D], F32)
        t2i_d = din("bt2img", [128, D], F32)
        wvec_d = din("wvec", [H, S], F32)
        lnw_d = din("lnwimg", [128, S], F32)
        out_d = dram.tile([QH, D], F32, kind="ExternalOutput", name="out",
                          uniquify=False)
        if debug:
            dbg = {}
            for nm, shp in [("dxc", [128, DC, L]), ("dsc1", [128, DC, 512]),
                            ("dsc3", [128, DC, 128]),
                            ("dagg", [128, DC, QH]), ("dx1", [128, 4, D]),
                            ("doun", [128, DC, QH])]:
                dbg[nm] = dram.tile(shp, F32R, kind="ExternalOutput",
                                    name=nm, uniquify=False)

        # ---- constants (live for the whole kernel) ----
        const = tc.alloc_tile_pool(name="const", bufs=1)
        bdc_sb = const.tile([128, DC], F32, name="bdc_sb")
        nc.sync.dma_start(bdc_sb[:], bdc_d[:])
        lnw_sb = const.tile([128, S], F32, name="lnw_sb")
        nc.sync.dma_start(lnw_sb[:], lnw_d[:])
        zcol = const.tile([128, 1], F32, name="zcol")
        nc.vector.memset(zcol[:], 0.0)
        epscol = const.tile([128, 1], F32, name="epscol")
        nc.vector.memset(epscol[:], EPS)

        # global weight-streaming + psum pools (whole kernel)
        wpool = tc.alloc_tile_pool(name="wpool", bufs=8)
        pspool = tc.alloc_tile_pool(name="pspool", bufs=4, space="PSUM")

        def wt_tile():
            return wpool.tile([128, 256], F32R, name="wt", bufs=8)

        def wtb_tile():
            return wpool.tile([128, 256], BF16, name="wtb", bufs=4)

        def wbig_tile(dt=F32R):
            return wpool.tile([128, 512], dt, name="wbig", bufs=8)

        def ps_t(shape=None):
            return pspool.tile(shape or [128, 512], F32, name="ps_t",
                               bufs=4)

        def ps_s(shape=None, dt=F32):
            return pspool.tile(shape or [128, 512], dt, name="ps_s",
                               bufs=2)

        def ps_o():
            return pspool.tile([128, QH], F32, name="ps_o", bufs=2)

        pid = nc.partition_id()
        q0 = (pid % 2) * QH

        # long-lived activation pools (stack: released near the end)
        pX1 = tc.alloc_tile_pool(name="pX1", bufs=1)
        x1 = pX1.tile([128, 4, D], F32R, name="x1")
        pAgg = tc.alloc_tile_pool(name="pAgg", bufs=1)
        agg = pAgg.tile([128, DC, QH], F32R, name="agg")

        pXC = tc.alloc_tile_pool(name="pXC", bufs=1)
        xc = pXC.tile([128, DC, L], BF16, name="xc")

        # ---------------- Phase A: dilated conv -> xc (feature-major) ---
        pA = tc.alloc_tile_pool(name="pA", bufs=1)
        xt_sb = pA.tile([128, DC, L + 4], BF16, name="xt_sb")
        nc.sync.dma_start(
            xt_sb[:], xt_d[:].rearrange("(c p) t -> p c t", p=128))
        for oj in range(4):
            paccs = [ps_t() for _ in range(4)]
            for k in range(3):
                for ic in range(DC):
                    wt = wtb_tile()
                    nc.sync.dma_start(wt[:], wdc_d[k, ic, oj])
                    first = (k == 0 and ic == 0)
                    last = (k == 2 and ic == DC - 1)
                    for half in range(2):
                        for tt in range(2):
                            nc.tensor.matmul(
                                paccs[half * 2 + tt][:],
                                wt[:, half * 128:half * 128 + 128],
                                xt_sb[:, ic, tt * 512 + 2 * k:
                                      tt * 512 + 2 * k + 512],
                                start=first, stop=last)
            for half in range(2):
                oc = oj * 2 + half
                for tt in range(2):
                    nc.scalar.activation(
                        xc[:, oc, tt * 512:tt * 512 + 512],
                        paccs[half * 2 + tt][:], AF.Relu,
                        bias=bdc_sb[:, oc:oc + 1])
        if debug:
            nc.sync.dma_start(dbg["dxc"][:], xc[:])
        pA.release()

        # ---------------- Phase B: pyramid (strided convs) --------------
        pSC = tc.alloc_tile_pool(name="pSC", bufs=1)
        sc1 = pSC.tile([128, DC, LS[1]], BF16, name="sc1")
        sc2 = pSC.tile([128, DC, LS[2]], BF16, name="sc2")
        sc3 = pSC.tile([128, DC, LS[3]], BF16, name="sc3")
        pB = tc.alloc_tile_pool(name="pB", bufs=1)
        srcs = [xc, sc1, sc2]
        dsts = [sc1, sc2, sc3]
        for s in range(3):
            src, dst = srcs[s], dsts[s]
            ln = LS[s + 1]
            for oj in range(4):
                paccs = [ps_t([128, ln]) for _ in range(2)]
                for k in range(CR):
                    for ic in range(DC):
                        wt = wtb_tile()
                        nc.sync.dma_start(wt[:], wdec_d[s, k, ic, oj])
                        first = (k == 0 and ic == 0)
                        last = (k == CR - 1 and ic == DC - 1)
                        for half in range(2):
                            nc.tensor.matmul(
                                paccs[half][:],
                                wt[:, half * 128:half * 128 + 128],
                                src[:, ic, k:2 * ln:2],
                                start=first, stop=last)
                for half in range(2):
                    nc.vector.tensor_copy(dst[:, oj * 2 + half, :],
                                          paccs[half][:])
        if debug:
            nc.sync.dma_start(dbg["dsc1"][:], sc1[:])
            nc.sync.dma_start(dbg["dsc3"][:], sc3[:])
        pB.release()

        # ---------------- Phase C: attention over 4 scales --------------
        pCt = tc.alloc_tile_pool(name="pCt", bufs=1)
        probsC = tc.alloc_tile_pool(name="probsC", bufs=4)
        tmpC = tc.alloc_tile_pool(name="tmpC", bufs=2)
        for s in range(S):
            src = [xc, sc1, sc2, sc3][s]
            ls = LS[s]
            kk_n = ls // 128
            nt_n = (ls + 511) // 512
            oun = pCt.tile([128, DC, QH], F32, name="oun")
            wv_tiles = None
            if debug and s == 0:
                dbg_oun_pending = True
            for g in range(4):      # head groups: heads 4g..4g+3
                oc0 = 2 * g        # d-chunks 2g, 2g+1 <-> q/k rows
                # --- queries for this group's chunks
                qs = pCt.tile([128, 2, QH], BF16, name="qs", bufs=2)
                pq = [ps_t([128, QH]) for _ in range(2)]
                for ic in range(DC):
                    wtq = wtb_tile()
                    nc.sync.dma_start(wtq[:], wq_d[s, ic, g])
                    for half in range(2):
                        nc.tensor.matmul(
                            pq[half][:],
                            wtq[:, half * 128:half * 128 + 128],
                            xc[:, ic, bass.ds(q0, QH)],
                            start=(ic == 0), stop=(ic == DC - 1))
                for half in range(2):
                    nc.vector.tensor_copy(qs[:, half, :], pq[half][:])
                # --- keys (feature-major) for chunks 2g, 2g+1
                kt = pCt.tile([128, 2, ls], BF16, name="kt")
                for nt in range(nt_n):
                    n = min(512, ls - nt * 512)
                    pk = [ps_t([128, n]) for _ in range(2)]
                    for ic in range(DC):
                        wtk = wtb_tile()
                        nc.sync.dma_start(wtk[:], wk_d[s, ic, g])
                        for half in range(2):
                            nc.tensor.matmul(
                                pk[half][:],
                                wtk[:, half * 128:half * 128 + 128],
                                src[:, ic, nt * 512:nt * 512 + n],
                                start=(ic == 0), stop=(ic == DC - 1))
                    for half in range(2):
                        nc.vector.tensor_copy(
                            kt[:, half, nt * 512:nt * 512 + n],
                            pk[half][:])
                # --- values (row-major, bf16, interleaved ones column)
                if g % 2 == 0:
                    wv_tiles = []
                    for ic in range(DC):
                        wt = wbig_tile(BF16)
                        nc.sync.dma_start(wt[:], wv_d[ic, g // 2])
                        wv_tiles.append(wt)
                va = pCt.tile([128, kk_n, 4 * 128], BF16, name="va",
                              bufs=2)
                nc.vector.memset(
                    va[:].rearrange("p k (h c) -> p k h c", c=128)
                    [:, :, :, 64:128], 1.0)
                for kk in range(kk_n):
                    pacc = ps_t([128, 256])
                    for ic in range(DC):
                        nc.tensor.matmul(
                            pacc[:],
                            src[:, ic, kk * 128:kk * 128 + 128],
                            wv_tiles[ic][:,
                                         (g % 2) * 256:(g % 2) * 256 + 256],
                            start=(ic == 0), stop=(ic == DC - 1))
                    nc.vector.tensor_copy(
                        va[:, kk, :].rearrange("p (h c) -> p h c", c=128)
                        [:, :, 0:64],
                        pacc[:].rearrange("p (h c) -> p h c", c=64))
                # --- attention for the 4 heads
                dstash = tmpC.tile([128, 2, QH], F32, name="dstash")
                for hh in range(4):
                    half, off = hh // 2, (hh % 2) * 64
                    pov = ps_o()
                    for kk in range(kk_n):
                        pss = ps_s([128, QH])
                        nc.tensor.matmul(
                            pss[:],
                            kt[off:off + 64, half, kk * 128:kk * 128 + 128],
                            qs[off:off + 64, half, :],
                            start=True, stop=True)
                        pb = probsC.tile([128, QH], BF16, name="pb")
                        nc.scalar.activation(pb[:], pss[:], AF.Exp,
                                             bias=zcol[:],
                                             scale=1.0 / np.sqrt(DH))
                        nc.tensor.matmul(
                            pov[0:128, :],
                            va[:, kk, hh * 128:hh * 128 + 128],
                            pb[:], start=(kk == 0), stop=(kk == kk_n - 1))
                    nc.vector.tensor_copy(
                        oun[off:off + 64, oc0 + half, :], pov[0:64, :])
                    nc.vector.tensor_copy(dstash[off:off + 64, half, :],
                                          pov[64:128, :])
                # R = w_s / den via exp(-ln(den) + ln(w_s)), batched on ACT
                Rg = tmpC.tile([128, 2, QH], F32, name="Rg", bufs=2)
                lnt = tmpC.tile([128, 2, QH], F32, name="lnt2")
                nc.scalar.activation(lnt[:], dstash[:], AF.Ln,
                                     bias=zcol[:])
                nc.scalar.activation(Rg[:], lnt[:], AF.Exp, scale=-1.0,
                                     bias=lnw_sb[:, s:s + 1])
                for j in range(2):
                    oc = oc0 + j
                    if s == 0:
                        nc.vector.tensor_tensor(
                            out=agg[:, oc, :], in0=oun[:, oc, :],
                            in1=Rg[:, j, :], op=OP.mult)
                    else:
                        t = tmpC.tile([128, QH], F32, name="tagg")
                        nc.vector.tensor_tensor(
                            out=t[:], in0=oun[:, oc, :],
                            in1=Rg[:, j, :], op=OP.mult)
                        nc.vector.tensor_tensor(
                            out=agg[:, oc, :], in0=agg[:, oc, :],
                            in1=t[:], op=OP.add)
            if debug and s == 0:
                nc.gpsimd.dma_start(dbg["doun"][:], oun[:])
        if debug:
            nc.sync.dma_start(dbg["dagg"][:], agg[:])
        tmpC.release()
        probsC.release()
        pCt.release()
        pSC.release()
        pXC.release()

        # ---------------- LN machinery + late constants -----------------
        pImgs = tc.alloc_tile_pool(name="pImgs", bufs=1)
        imgs = {}
        for nm, d in [("b2", b2i_d), ("g1", g1i_d), ("t1", t1i_d),
                      ("g2", g2i_d), ("t2", t2i_d)]:
            t = pImgs.tile([128, D], F32, name=f"img_{nm}")
            nc.sync.dma_start(t[:], d[:])
            imgs[nm] = t
        b1_sb = pImgs.tile([128, FC], F32, name="b1_sb")
        nc.sync.dma_start(b1_sb[:], b1_d[:])
        pLN = tc.alloc_tile_pool(name="pLN", bufs=2)
        tmpLN = tc.alloc_tile_pool(name="tmpLN", bufs=3)

        def layer_norm(xin, xout, g_img, t_img):
            # xin/xout: [128, D] APs; row-wise LN along free dim
            musum = tmpLN.tile([128, 1], F32, name="lnmu")
            nc.vector.reduce_sum(musum[:], xin, axis=mybir.AxisListType.X)
            negmu = tmpLN.tile([128, 1], F32, name="lnnm")
            nc.scalar.mul(negmu[:], musum[:], -1.0 / D)
            xm = pLN.tile([128, D], F32, name="lnxm")
            nc.scalar.activation(xm[:], xin, AF.Identity, bias=negmu[:])
            sq = pLN.tile([128, D], F32, name="lnsq")
            ssq = tmpLN.tile([128, 1], F32, name="lnss")
            nc.scalar.activation(sq[:], xm[:], AF.Square, bias=zcol[:],
                                 accum_out=ssq[:])
            std = tmpLN.tile([128, 1], F32, name="lnsd")
            nc.scalar.activation(std[:], ssq[:], AF.Sqrt, scale=1.0 / D,
                                 bias=epscol[:])
            rstd = tmpLN.tile([128, 1], F32, name="lnrs")
            nc.vector.reciprocal(rstd[:], std[:])
            t = pLN.tile([128, D], F32, name="lnt")
            nc.vector.scalar_tensor_tensor(
                out=t[:], in0=xm[:], scalar=rstd[:, 0:1], in1=g_img[:],
                op0=OP.mult, op1=OP.mult)
            nc.vector.tensor_tensor(out=xout, in0=t[:], in1=t_img[:],
                                    op=OP.add)

        # ---------------- Phase D: Wo + residual + LN1 ------------------
        pD = tc.alloc_tile_pool(name="pD", bufs=1)
        xres_sb = pD.tile([128, 4, D], F32, name="xres_sb")
        nc.sync.dma_start(
            xres_sb[:], xres_d[:].rearrange("(c p) d -> p c d", p=128))
        xpms = [pD.tile([128, D], F32, name=f"xpm{i}") for i in range(4)]
        aggb = pD.tile([128, DC, QH], BF16, name="aggb")
        nc.vector.tensor_copy(aggb[:], agg[:])
        for os_ in range(2):
            pms = [ps_t() for _ in range(4)]
            for dc in range(DC):
                wt = wbig_tile(BF16)
                nc.sync.dma_start(wt[:], wo_d[dc, os_])
                for qc in range(4):
                    nc.tensor.matmul(
                        pms[qc][:], aggb[:, dc, qc * 128:qc * 128 + 128],
                        wt[:], start=(dc == 0), stop=(dc == DC - 1))
            for qc in range(4):
                nc.vector.tensor_tensor(
                    out=xpms[qc][:, os_ * 512:os_ * 512 + 512],
                    in0=pms[qc][:],
                    in1=xres_sb[:, qc, os_ * 512:os_ * 512 + 512],
                    op=OP.add)
        for qc in range(4):
            layer_norm(xpms[qc][:], x1[:, qc, :], imgs["g1"], imgs["t1"])
        if debug:
            nc.sync.dma_start(dbg["dx1"][:], x1[:])
        pD.release()

        # ---------------- Phase E+F: transpose, FFN ---------------------
        pEF = tc.alloc_tile_pool(name="pEF", bufs=1)
        tfF = tc.alloc_tile_pool(name="tfF", bufs=2)
        x1t = pEF.tile([128, DC, QH], BF16, name="x1t")
        for qc in range(4):
            x1b = tfF.tile([128, D], BF16, name="x1b")
            nc.vector.tensor_copy(x1b[:], x1[:, qc, :])
            nc.sync.dma_start_transpose(
                x1t[:, :, qc * 128:qc * 128 + 128], x1b[:])

        hT = pEF.tile([128, FC, QH], BF16, name="hT")
        for fj in range(16):
            paccs = [ps_t([128, QH]) for _ in range(2)]
            for ic in range(DC):
                wt = wtb_tile()
                nc.sync.dma_start(wt[:], w1_d[ic, fj])
                for half in range(2):
                    nc.tensor.matmul(
                        paccs[half][:], wt[:, half * 128:half * 128 + 128],
                        x1t[:, ic, :], start=(ic == 0), stop=(ic == DC - 1))
            for half in range(2):
                fc = fj * 2 + half
                nc.scalar.activation(hT[:, fc, :], paccs[half][:], AF.Gelu,
                                     bias=b1_sb[:, fc:fc + 1])

        pF2 = tc.alloc_tile_pool(name="pF2", bufs=1)
        x2s = [pF2.tile([128, D], F32, name=f"x2_{i}") for i in range(4)]
        for os_ in range(2):
            pfs = [ps_t() for _ in range(4)]
            for fc in range(FC):
                wt = wbig_tile(BF16)
                nc.sync.dma_start(wt[:], w2_d[fc, os_])
                for qc in range(4):
                    nc.tensor.matmul(
                        pfs[qc][:], hT[:, fc, qc * 128:qc * 128 + 128],
                        wt[:], start=(fc == 0), stop=(fc == FC - 1))
            for qc in range(4):
                t = tfF.tile([128, 512], F32, name="tf")
                nc.vector.tensor_tensor(
                    out=t[:], in0=pfs[qc][:],
                    in1=imgs["b2"][:, os_ * 512:os_ * 512 + 512], op=OP.add)
                nc.vector.tensor_tensor(
                    out=x2s[qc][:, os_ * 512:os_ * 512 + 512], in0=t[:],
                    in1=x1[:, qc, os_ * 512:os_ * 512 + 512], op=OP.add)
        for qc in range(4):
            outq = tfF.tile([128, D], F32, name="outq")
            layer_norm(x2s[qc][:], outq[:], imgs["g2"], imgs["t2"])
            nc.sync.dma_start(
                out_d[:].rearrange("(c p) d -> p c d", p=128)[:, qc, :],
                outq[:])

        for p in (pF2, tfF, pEF, tmpLN, pLN, pImgs, pAgg, pX1,
                  pspool, wpool, const, dram):
            p.release()
    nc.compile()
    return nc


def _chunk_lhst(w, dt=np.float32):
    # [I, O] -> [I/128, O/256, 128, 256] contiguous blocks
    i, o = w.shape
    return np.ascontiguousarray(
        w.reshape(i // 128, 128, o // 256, 256).transpose(0, 2, 1, 3)
    ).astype(dt, copy=False)


def _chunk_rhs(w):
    # [I, O] -> [I/128, O/512, 128, 512]
    i, o = w.shape
    return np.ascontiguousarray(
        w.reshape(i // 128, 128, o // 512, 512).transpose(0, 2, 1, 3))


def _chunk_rhs_bf16(w):
    return _chunk_rhs(w).astype(ml_dtypes.bfloat16)


def _prep(inputs):
    f = {k: np.asarray(v, dtype=np.float32) for k, v in inputs.items()}
    W_dc, W_dec = f["W_dc"], f["W_dec"]
    bf = ml_dtypes.bfloat16
    wdc = np.stack([_chunk_lhst(W_dc[:, :, k].T, bf) for k in range(3)])
    wdec = np.stack([
        np.stack([_chunk_lhst(W_dec[s, :, :, k].T, bf) for k in range(CR)])
        for s in range(3)])
    wq = np.stack([_chunk_lhst(f["Wq"][s], bf) for s in range(S)])
    wk = np.stack([_chunk_lhst(f["Wk"][s], bf) for s in range(S)])

    ex = np.exp(f["agg_logits"] - f["agg_logits"].max())
    w = (ex / ex.sum()).astype(np.float32)
    aux = np.float32(-(w * np.log(w + np.float32(1e-9))).sum())

    shared = {
        "wdc": wdc,
        "bdc": np.ascontiguousarray(f["b_dc"].reshape(DC, 128).T),
        "wdec": wdec, "wq": wq, "wk": wk,
        "wv": _chunk_rhs_bf16(f["Wv"]), "wo": _chunk_rhs_bf16(f["Wo"]),
        "w1": _chunk_lhst(f["W1"], ml_dtypes.bfloat16),
        "b1": np.ascontiguousarray(f["b1"].reshape(FC, 128).T),
        "w2": _chunk_rhs_bf16(f["W2"]),
        "b2img": np.ascontiguousarray(
            np.broadcast_to(f["b2"], (128, D))),
        "g1img": np.ascontiguousarray(np.broadcast_to(f["gamma1"], (128, D))),
        "bt1img": np.ascontiguousarray(np.broadcast_to(f["beta1"], (128, D))),
        "g2img": np.ascontiguousarray(np.broadcast_to(f["gamma2"], (128, D))),
        "bt2img": np.ascontiguousarray(np.broadcast_to(f["beta2"], (128, D))),
        "wvec": np.ascontiguousarray(np.broadcast_to(w, (H, S))),
        "lnwimg": np.ascontiguousarray(
            np.broadcast_to(np.log(w), (128, S)).astype(np.float32)),
    }
    in_maps = []
    x = f["x"]
    for c in range(NC):
        b, half = c // 2, c % 2
        xt = np.zeros((D, L + 4), np.float32)
        xt[:, 2:L + 2] = x[b].T
        m = dict(shared)
        m["xt_pad"] = xt.astype(ml_dtypes.bfloat16)
        m["xres"] = np.ascontiguousarray(x[b, half * QH:(half + 1) * QH, :])
        in_maps.append(m)
    return in_maps, aux


def run(inputs, trace=False, debug=False):
    key = ("ncd" if debug else "nc")
    if key not in _CACHE:
        _CACHE[key] = _build(debug=debug)
    nc = _CACHE[key]
    in_maps, aux = _prep(inputs)
    res = run_bass_kernel_spmd(nc, in_maps, core_ids=list(range(NC)),
                               trace=trace)
    out = np.empty((B, L, D), np.float32)
    for c in range(NC):
        b, half = c // 2, c % 2
        out[b, half * QH:(half + 1) * QH, :] = res.results[c]["out"]
    return (out, aux), res


def kernel(**inputs):
    (out, aux), _ = run(inputs, trace=False)
    return out, aux
